# revision 15
# baseline (speedup 1.0000x reference)
"""Trainium2 Bass kernel for multi-head causal attention (fp8 DoubleRow).

Problem: B=2, S=2048, D=1024, H=16, DH=64 (fp32), causal attention with
QKV projections and output projection summed over heads.

Sharding: 8 cores = (batch b in {0,1}) x (head-group hg in {0..3}, 4 heads
each).  Each core computes a partial output sum over its 4 heads for its
batch; the host sums the 4 partials per batch and adds b_O.

Performance design (CoreSim cost model: matmul = out_free x cyc/row x
PE_CYCLE; fp8e4 DoubleRow = 0.5 cyc/row with 2 contraction k-tiles per
instruction; Act exp = 0.833 ns/col and is the end-to-end bottleneck):

  - All projection / score / PV matmuls run as fp8e4 DoubleRow.  fp8e4m3
    has too few mantissa bits for full accuracy, so each matmul pairs its
    two k-tile slots to recover precision where it matters:
      * q/k proj: x8 (x) (SC*w)8 over dc-chunk pairs -- plain fp8 (cheap);
        weights are pre-scaled by SC=32 on the host because |W|~0.02 falls
        into e4m3's subnormal range unscaled.  (1/SC folds into W_O; the
        extra SC^2 in the scores folds into the exp's scale argument.)
      * v proj: 3-term hi/lo: (xh+xl)@wh + xh@wl, so v is accurate to
        ~0.2% (v is the most error-sensitive operand: early rows average
        few v's, so v errors pass straight to the output).
      * scores: stationary k8 duplicated in both slots, moving q split
        hi+lo across the slots -> only k's single-fp8 rounding remains.
      * PV: stationary (v_hi, v_lo) slot pair, moving pt8 read twice via a
        stride-0 AP.  64 ones columns (slot0=1, slot1=0) next to each
        head's v columns make rows 64..127 of the PV psum the softmax
        denominator l (free: stationary columns don't cost cycles).
  - exp(scale*s) with scale=1/SC^2 runs on Act reading PSUM score strips
    [128 x <=1024]; Act does nothing else (no table thrash, ~72us busy).
  - biases enter via tiny rank-1 fp8 DR matmuls (row 0 of a host-built
    constant) accumulated into each projection psum -- no extra DVE work
    and exact for the all-zero biases of this problem up to fp8 rounding.
  - DVE handles every PSUM-reading pointwise op (q/k/v fp8 writes + split
    residuals, reciprocal, zT normalize); gpsimd (Pool) handles SBUF-only
    work (diag masks, ones memsets) + half the DMA queue load.
  - output projection psum is DMA'd straight to DRAM as fp32 partials
    (alternating sync/pool queues); the host sums partials + b_O.

Measured end-to-end relative error vs the fp32 reference: ~1.3e-2
(dominated by pt8 rounding; tolerance 2e-2).

A BIR post-processing patch (installed on import) hoists excess sync waits
off instructions into standalone EventSemaphore ops -- walrus codegen allows
only 1 wait on the fused 4-byte-weight-load matmul encoding and few on
other opcodes, and Tile emits more.
"""

import sys

import numpy as np

for _p in ("/opt/trn_rl_repo",):
    if _p not in sys.path:
        sys.path.insert(0, _p)

import ml_dtypes

import concourse.bass as bass
import concourse.tile as tile
from concourse import mybir
from concourse.bass_utils import run_bass_kernel_spmd


def _hoist_matmul_waits(bir_json: bytes) -> bytes:
    """Move extra sync waits off instructions into EventSemaphore ops."""
    import orjson

    m = orjson.loads(bir_json)
    changed = False
    for fn in m.get("functions", []):
        for bb in fn.get("blocks", []):
            insts = bb.get("instructions", [])
            out = []
            for inst in insts:
                si = inst.get("sync_info") or {}
                waits = si.get("on_wait") or []
                if len(waits) > 1:
                    keep = waits[-1]
                    for wi, w in enumerate(waits[:-1]):
                        out.append({
                            "debug": inst.get("debug", 0),
                            "engine": inst["engine"],
                            "ins": [],
                            "name": f"{inst['name']}-hw{wi}",
                            "opcode": "EventSemaphore",
                            "outs": [],
                            "sync_info": {"on_update": [],
                                          "on_wait": [w]},
                        })
                    si["on_wait"] = [keep]
                    inst["sync_info"] = si
                    changed = True
                out.append(inst)
            bb["instructions"] = out
    if not changed:
        return bir_json
    return orjson.dumps(m)


def _install_bir_patch():
    from concourse import bass2jax as _b2j
    from concourse import bass_utils as _bu

    if getattr(_b2j, "_mm_wait_patch", False):
        return

    _orig = _bu.compile_bir_kernel

    def _patched(bir_json, tmpdir, neff_name="file.neff"):
        return _orig(_hoist_matmul_waits(bir_json), tmpdir, neff_name)

    _b2j.compile_bir_kernel = _patched
    _bu.compile_bir_kernel = _patched
    _b2j._mm_wait_patch = True


_install_bir_patch()

# Problem dims (hardcoded per harness contract).
B, S, D, H, DH = 2, 2048, 1024, 16, 64
ATTN_SCALE = 8.0
NCORES = 8
HL = H // (NCORES // B)  # 4 local heads per core
E = HL * DH              # 256 local head dims
P = 128
DC = D // P              # 8 contraction chunks
EC = E // P              # 2 e-chunks (head pairs)
NSB = S // P             # 16 s-blocks of 128
NI = 1024                # i-group width for score strips
NG = S // NI             # 2 i-groups
SC = 32.0                # host weight pre-scale (fp8 subnormal dodge)
F32 = mybir.dt.float32
F32R = mybir.dt.float32r
F16 = mybir.dt.float16
F8 = mybir.dt.float8e4
AF = mybir.ActivationFunctionType
DR = mybir.MatmulPerfMode.DoubleRow
NPF8 = ml_dtypes.float8_e4m3


def _round_f32r(arr):
    from neuronxcc.starfish.support import dtype as nxd
    a = np.ascontiguousarray(np.asarray(arr, dtype=np.float32))
    return np.asarray(nxd.static_cast(a, dtype=nxd.float32r)).view(np.float32)


def _dup_t(ap):
    """Insert a stride-0 size-2 dim after the partition dim (DoubleRow
    k-tile duplication without a second copy in SBUF)."""
    return bass.AP(tensor=ap.tensor, offset=ap.offset,
                   ap=[list(ap.ap[0]), [0, 2]] + [list(x) for x in ap.ap[1:]])


def _emit(ctx, tc, xq8, xk8, xv8, wq8, wk8, wv8, wo, bqk8, ones8, bv8,
          masks8, out):
    nc = tc.nc

    persist = ctx.enter_context(tc.tile_pool(name="persist", bufs=1))
    xstage = ctx.enter_context(tc.tile_pool(name="xstage", bufs=4))
    xvstage = ctx.enter_context(tc.tile_pool(name="xvstage", bufs=2))
    ptpool = ctx.enter_context(tc.tile_pool(name="ptp", bufs=10))
    outpool = ctx.enter_context(tc.tile_pool(name="outp", bufs=4))
    small = ctx.enter_context(tc.tile_pool(name="small", bufs=6))
    # PSUM budget (8 banks of [128, 2KB]):
    #   ps_s: score strips [128, 1024] f32 = 2 banks x 2 bufs = 4
    #   ps_z: PV accumulators [128, 512] = 1 bank x 2 bufs = 2
    #   ps_mm: proj / outproj [128, <=512] = 1 bank x 2 bufs = 2
    ps_s = ctx.enter_context(tc.tile_pool(name="ps_s", bufs=2, space="PSUM"))
    ps_z = ctx.enter_context(tc.tile_pool(name="ps_z", bufs=2, space="PSUM"))
    ps_mm = ctx.enter_context(tc.tile_pool(name="ps_mm", bufs=2, space="PSUM"))

    # --- persistent activations ---
    # kT8/qT8: partitions hb*64+e for head (hc, hb); dims [hc, kslot, col]
    kT8 = persist.tile([P, EC, 2, S], F8)
    qT8_g = [persist.tile([P, EC, 2, NI], F8, name=f"qT{g}") for g in range(NG)]
    # v8: [j-part, sb, slot(hi/lo), 4h x (64 v + 64 ones)]
    v8_g = [persist.tile([P, NSB // NG, 2, HL * P], F8, name=f"v{g}")
            for g in range(NG)]
    zT = persist.tile([P, EC, S], F32R)

    wq_sb = persist.tile([P, DC, E], F8)
    wk_sb = persist.tile([P, DC, E], F8)
    wv_sb = persist.tile([P, DC, 2, E], F8)
    wo_sb = persist.tile([P, EC, D], F32R)
    bqk_sb = persist.tile([P, 2, 4, P], F8)
    ones_sb = persist.tile([P, 2, 512], F8)
    bv_sb = persist.tile([P, 2, E], F8)
    masks_sb = persist.tile([P, P], F8)

    xq_r = xq8.rearrange("(c p) s -> p c s", p=P)
    xk_r = xk8.rearrange("(c p) s -> p c s", p=P)
    xv_r = xv8.rearrange("p (c t) s -> p c t s", t=2)

    def emit_kq(g):
        if g == 0:
            nc.sync.dma_start(out=wk_sb,
                              in_=wk8.rearrange("(c p) e -> p c e", p=P))
            nc.sync.dma_start(out=bqk_sb, in_=bqk8)
            nc.sync.dma_start(out=ones_sb, in_=ones8)
        for nl in range(NI // 512):
            n = g * (NI // 512) + nl
            cols = slice(n * 512, (n + 1) * 512)
            lcols = slice(nl * 512, (nl + 1) * 512)
            # ---- K chunk ----
            xs = xstage.tile([P, DC, 512], F8, tag="xs")
            nc.sync.dma_start(out=xs, in_=xk_r[:, :, cols])
            if g == 0 and nl == 0:
                nc.sync.dma_start(
                    out=wq_sb, in_=wq8.rearrange("(c p) e -> p c e", p=P))
            for m in range(EC):
                ps = ps_mm.tile([P, 512], F32, tag="mm")
                nc.tensor.matmul(ps, lhsT=bqk_sb[:, :, 2 + m, :],
                                 rhs=ones_sb, perf_mode=DR,
                                 start=True, stop=False)
                for dp in range(DC // 2):
                    nc.tensor.matmul(
                        ps,
                        lhsT=wk_sb[:, 2 * dp:2 * dp + 2, m * P:(m + 1) * P],
                        rhs=xs[:, 2 * dp:2 * dp + 2, :],
                        perf_mode=DR,
                        start=False, stop=(dp == DC // 2 - 1))
                # k8 duplicated into both DoubleRow slots
                nc.vector.tensor_copy(out=kT8[:, m, 0, cols], in_=ps)
                nc.vector.tensor_copy(out=kT8[:, m, 1, cols], in_=ps)
            # ---- Q chunk ----
            xs = xstage.tile([P, DC, 512], F8, tag="xs")
            nc.sync.dma_start(out=xs, in_=xq_r[:, :, cols])
            for m in range(EC):
                ps = ps_mm.tile([P, 512], F32, tag="mm")
                nc.tensor.matmul(ps, lhsT=bqk_sb[:, :, m, :],
                                 rhs=ones_sb, perf_mode=DR,
                                 start=True, stop=False)
                for dp in range(DC // 2):
                    nc.tensor.matmul(
                        ps,
                        lhsT=wq_sb[:, 2 * dp:2 * dp + 2, m * P:(m + 1) * P],
                        rhs=xs[:, 2 * dp:2 * dp + 2, :],
                        perf_mode=DR,
                        start=False, stop=(dp == DC // 2 - 1))
                # q_hi = fp8(ps/8); q_lo = fp8(ps/8 - q_hi)
                nc.vector.tensor_scalar(
                    out=qT8_g[g][:, m, 0, lcols], in0=ps,
                    scalar1=1.0 / ATTN_SCALE, scalar2=None,
                    op0=mybir.AluOpType.mult)
                nc.vector.scalar_tensor_tensor(
                    out=qT8_g[g][:, m, 1, lcols], in0=ps,
                    scalar=1.0 / ATTN_SCALE,
                    in1=qT8_g[g][:, m, 0, lcols],
                    op0=mybir.AluOpType.mult,
                    op1=mybir.AluOpType.subtract)

    def emit_v(g):
        if g == 0:
            nc.gpsimd.dma_start(out=wv_sb,
                                in_=wv8.rearrange("p (c t) e -> p c t e", t=2))
            nc.gpsimd.dma_start(out=bv_sb, in_=bv8)
            nc.gpsimd.dma_start(out=masks_sb, in_=masks8)
        else:
            nc.sync.dma_start(out=wo_sb,
                              in_=wo.rearrange("(c p) d -> p c d", p=P))
        # ones slots once per g: slot0 = 1.0, slot1 = 0.0
        ones_ap0 = v8_g[g][:, :, 0, :].rearrange(
            "p s (h c) -> p s h c", h=HL)[:, :, :, DH:P]
        ones_ap1 = v8_g[g][:, :, 1, :].rearrange(
            "p s (h c) -> p s h c", h=HL)[:, :, :, DH:P]
        nc.gpsimd.memset(ones_ap0, 1.0)
        nc.gpsimd.memset(ones_ap1, 0.0)
        for half in range(2):
            n = g * 2 + half
            xv_st = xvstage.tile([P, DC, 2, 512], F8, tag="xv")
            nc.gpsimd.dma_start(out=xv_st,
                                in_=xv_r[:, :, :, n * 512:(n + 1) * 512])
            for sbl in range(4):
                sb_l = half * 4 + sbl
                scols = slice(sbl * P, (sbl + 1) * P)
                ps = ps_mm.tile([P, E], F32, tag="mm")
                nc.tensor.matmul(ps, lhsT=ones_sb[:, :, 0:P], rhs=bv_sb,
                                 perf_mode=DR, start=True, stop=False)
                for dc in range(DC):
                    nc.tensor.matmul(
                        ps,
                        lhsT=xv_st[:, dc, :, scols],
                        rhs=_dup_t(wv_sb[:, dc, 0, :]),
                        perf_mode=DR, start=False, stop=False)
                for dp in range(DC // 2):
                    nc.tensor.matmul(
                        ps,
                        lhsT=xv_st[:, 2 * dp:2 * dp + 2, 0, scols],
                        rhs=wv_sb[:, 2 * dp:2 * dp + 2, 1, :],
                        perf_mode=DR,
                        start=False, stop=(dp == DC // 2 - 1))
                ps_r = ps.rearrange("p (h e) -> p h e", h=HL)
                hi_ap = v8_g[g][:, sb_l, 0, :].rearrange(
                    "p (h c) -> p h c", h=HL)[:, :, 0:DH]
                lo_ap = v8_g[g][:, sb_l, 1, :].rearrange(
                    "p (h c) -> p h c", h=HL)[:, :, 0:DH]
                nc.vector.tensor_copy(out=hi_ap, in_=ps_r)
                nc.vector.scalar_tensor_tensor(
                    out=lo_ap, in0=ps_r, scalar=1.0, in1=hi_ap,
                    op0=mybir.AluOpType.mult,
                    op1=mybir.AluOpType.subtract)

    def emit_attn(g):
        jmax = 8 * g + 8
        for h in range(HL):
            hc, hb = h // 2, h % 2
            p0 = hb * DH

            def _ct(jb):
                t = jb - 8 * g
                return 0 if t < 4 else 1

            contrib = [[jb for jb in range(jmax) if _ct(jb) <= c]
                       for c in range(2)]
            zps = [ps_z.tile([P, 512], F32, tag="z", name=f"zps{c}")
                   for c in range(2)]
            for jb in range(jmax):
                t = jb - 8 * g
                ct = _ct(jb)
                sps = ps_s.tile([P, NI], F32, tag="s")
                pt = ptpool.tile([P, NI], F8, tag="pt")
                zlo = max(0, t) * P
                for c in range(ct, 2):
                    c0 = c * 512
                    lo = max(zlo, c0)
                    nc.tensor.matmul(
                        sps[:, lo:c0 + 512],
                        lhsT=kT8[p0:p0 + DH, hc, :, jb * P:(jb + 1) * P],
                        rhs=qT8_g[g][p0:p0 + DH, hc, :, lo:c0 + 512],
                        perf_mode=DR, start=True, stop=True)
                nc.scalar.activation(out=pt[:, zlo:NI], in_=sps[:, zlo:NI],
                                     func=AF.Exp, scale=1.0 / (SC * SC))
                if t >= 0:
                    nc.gpsimd.tensor_mul(
                        out=pt[:, zlo:zlo + P],
                        in0=pt[:, zlo:zlo + P],
                        in1=masks_sb)
                for c in range(ct, 2):
                    c0 = c * 512
                    lo = max(zlo, c0)
                    nc.tensor.matmul(
                        zps[c][:, lo - c0:512],
                        lhsT=v8_g[jb // 8][:, jb % 8, :, h * P:(h + 1) * P],
                        rhs=_dup_t(pt[:, lo:c0 + 512]),
                        perf_mode=DR,
                        start=(jb == contrib[c][0]),
                        stop=(jb == contrib[c][-1]))
            for c in range(2):
                bcr = small.tile([DH, 512], F32, tag="bcr")
                nc.vector.reciprocal(bcr, zps[c][DH:2 * DH, :])
                icol = g * NI + c * 512
                nc.vector.tensor_mul(
                    out=zT[p0:p0 + DH, hc, icol:icol + 512],
                    in0=zps[c][0:DH, :],
                    in1=bcr)
        # output projection: psum -> fp16 SBUF (DVE; Act for the tail where
        # exp is already done) -> DRAM
        for ib in range(8 * g, 8 * g + 8):
            osb = outpool.tile([P, D], F16, tag="o")
            for d2 in range(2):
                ops = ps_mm.tile([P, 512], F32, tag="mm")
                for ec in range(EC):
                    nc.tensor.matmul(
                        ops,
                        lhsT=zT[:, ec, ib * P:(ib + 1) * P],
                        rhs=wo_sb[:, ec, d2 * 512:(d2 + 1) * 512],
                        start=(ec == 0),
                        stop=(ec == EC - 1))
                if g == 1 and d2 == 1:
                    nc.scalar.activation(
                        out=osb[:, d2 * 512:(d2 + 1) * 512], in_=ops,
                        func=AF.Copy)
                else:
                    nc.vector.tensor_copy(
                        out=osb[:, d2 * 512:(d2 + 1) * 512], in_=ops)
            eng = nc.gpsimd if ib % 2 == 0 else nc.sync
            eng.dma_start(out=out[ib * P:(ib + 1) * P, :], in_=osb)

    emit_kq(0)
    emit_v(0)
    emit_kq(1)
    emit_v(1)
    # wo load early enough for outproj(0) but after the x loads
    emit_attn(0)
    emit_attn(1)


def build_nc():
    from contextlib import ExitStack

    nc = bass.Bass()
    xq8 = nc.dram_tensor("xq8", [D, S], F8, kind="ExternalInput")[:]
    xk8 = nc.dram_tensor("xk8", [D, S], F8, kind="ExternalInput")[:]
    xv8 = nc.dram_tensor("xv8", [P, 2 * DC, S], F8, kind="ExternalInput")[:]
    wq8 = nc.dram_tensor("wq8", [D, E], F8, kind="ExternalInput")[:]
    wk8 = nc.dram_tensor("wk8", [D, E], F8, kind="ExternalInput")[:]
    wv8 = nc.dram_tensor("wv8", [P, 2 * DC, E], F8, kind="ExternalInput")[:]
    wo = nc.dram_tensor("wo", [E, D], F32R, kind="ExternalInput")[:]
    bqk8 = nc.dram_tensor("bqk8", [P, 2, 4, P], F8, kind="ExternalInput")[:]
    ones8 = nc.dram_tensor("ones8", [P, 2, 512], F8, kind="ExternalInput")[:]
    bv8 = nc.dram_tensor("bv8", [P, 2, E], F8, kind="ExternalInput")[:]
    masks8 = nc.dram_tensor("masks8", [P, P], F8, kind="ExternalInput")[:]
    out = nc.dram_tensor("out", [S, D], F16, kind="ExternalOutput")[:]
    with tile.TileContext(nc) as tc:
        with ExitStack() as ctx:
            _emit(ctx, tc, xq8, xk8, xv8, wq8, wk8, wv8, wo, bqk8, ones8,
                  bv8, masks8, out)
    return nc


_CACHE = {}


def _get_nc():
    if "nc" not in _CACHE:
        _CACHE["nc"] = build_nc()
    return _CACHE["nc"]


def _f8(a):
    return np.asarray(a, dtype=np.float32).astype(NPF8)


def make_in_maps(query_input, key_input, value_input, W_Q, W_K, W_V, W_O,
                 b_Q, b_K, b_V, b_O):
    qi = np.asarray(query_input, dtype=np.float32)
    ki = np.asarray(key_input, dtype=np.float32)
    vi = np.asarray(value_input, dtype=np.float32)
    W_Q = np.asarray(W_Q, dtype=np.float32)
    W_K = np.asarray(W_K, dtype=np.float32)
    W_V = np.asarray(W_V, dtype=np.float32)
    W_O = np.asarray(W_O, dtype=np.float32)
    b_Q = np.asarray(b_Q, dtype=np.float32)
    b_K = np.asarray(b_K, dtype=np.float32)
    b_V = np.asarray(b_V, dtype=np.float32)

    masks8 = np.triu(np.ones((P, P), dtype=np.float32)).astype(NPF8)
    ones8 = np.zeros((P, 2, 512), dtype=NPF8)
    ones8[0, 0, :] = 1.0

    xT = {}
    for b in range(B):
        xT[("q", b)] = _f8(qi[b].T)
        xT[("k", b)] = _f8(ki[b].T)
        vT = np.ascontiguousarray(vi[b].T)
        vh = _f8(vT)
        vl = _f8(vT - vh.astype(np.float32))
        # [P, 2*DC, S]: index (p, 2*dc+t, s) = component t of row dc*128+p
        vs = np.stack([vh.reshape(DC, P, S), vl.reshape(DC, P, S)], axis=2)
        xT[("v", b)] = np.ascontiguousarray(
            vs.transpose(1, 0, 2, 3).reshape(P, 2 * DC, S))

    in_maps = []
    for core in range(NCORES):
        b, hg = core // (NCORES // B), core % (NCORES // B)
        hs = slice(hg * HL, (hg + 1) * HL)
        Wq = np.transpose(W_Q[hs], (1, 0, 2)).reshape(D, E)
        Wk = np.transpose(W_K[hs], (1, 0, 2)).reshape(D, E)
        Wv = np.ascontiguousarray(
            np.transpose(W_V[hs], (1, 0, 2)).reshape(D, E)) * SC
        wvh = _f8(Wv)
        wvl = _f8(Wv - wvh.astype(np.float32))
        wvs = np.stack([wvh.reshape(DC, P, E), wvl.reshape(DC, P, E)], axis=2)
        wv8_host = np.ascontiguousarray(
            wvs.transpose(1, 0, 2, 3).reshape(P, 2 * DC, E))
        bqk8 = np.zeros((P, 2, 4, P), dtype=NPF8)
        bq_f = (SC * b_Q[hs].reshape(E)).astype(np.float32)
        bk_f = (SC * b_K[hs].reshape(E)).astype(np.float32)
        bqk8[0, 0, 0, :] = bq_f[0:P].astype(NPF8)
        bqk8[0, 0, 1, :] = bq_f[P:E].astype(NPF8)
        bqk8[0, 0, 2, :] = bk_f[0:P].astype(NPF8)
        bqk8[0, 0, 3, :] = bk_f[P:E].astype(NPF8)
        bv8 = np.zeros((P, 2, E), dtype=NPF8)
        bv8[0, 0, :] = (SC * b_V[hs].reshape(E)).astype(NPF8)
        in_maps.append({
            "xq8": xT[("q", b)],
            "xk8": xT[("k", b)],
            "xv8": xT[("v", b)],
            "wq8": _f8(SC * Wq),
            "wk8": _f8(SC * Wk),
            "wv8": wv8_host,
            "wo": _round_f32r(W_O[hs].reshape(E, D) / SC),
            "bqk8": bqk8,
            "ones8": ones8,
            "bv8": bv8,
            "masks8": masks8,
        })
    return in_maps


def gather_out(results, b_O):
    out = np.zeros((B, S, D), dtype=np.float64)
    for core in range(NCORES):
        out[core // (NCORES // B)] += results[core]["out"].astype(np.float64)
    out += np.asarray(b_O, dtype=np.float64)
    return out.astype(np.float32)


def kernel(query_input, key_input, value_input, W_Q, W_K, W_V, W_O,
           b_Q, b_K, b_V, b_O):
    nc = _get_nc()
    in_maps = make_in_maps(query_input, key_input, value_input,
                           W_Q, W_K, W_V, W_O, b_Q, b_K, b_V, b_O)
    res = run_bass_kernel_spmd(nc, in_maps, list(range(NCORES)))
    return gather_out(res.results, b_O)


def kernel_timed(inputs, trace_cores=None, **kwargs):
    """Like kernel() but traces and returns (out, BassKernelResults)."""
    nc = _get_nc()
    in_maps = make_in_maps(**inputs)
    res = run_bass_kernel_spmd(
        nc, in_maps, list(range(NCORES)), trace=True,
        trace_cores=trace_cores, **kwargs)
    return gather_out(res.results, inputs["b_O"]), res
